# revision 22
# baseline (speedup 1.0000x reference)
"""Trainium2 Bass kernel for ClippingAttentionEngine.

Sharding: core c -> (batch b = c//2, head-group hg = c%2, 8 heads each).
Each core computes Q/K/V projections for its 8 heads, attention, and the
partial transposed output projection over its head slice; host sums the two
per-batch partials, transposes, and adds the constant bias terms
(bo + bv @ Wo^T -- the V bias passes through softmax averaging unchanged).

The per-sample sparse/dense branch is folded into a single dense-shaped
program via a MULTIPLICATIVE prior M (host-built, bf16):
  dense batch:  M[q,k] = pm[q,k] ? 1 : exp(-lambda)
  sparse batch: M[q,k] = multiplicity of key k in prior_indices[q] (masked
                slots excluded), so P = exp(s) * M reproduces the gathered
                sparse softmax exactly (duplicates included, 0 = exact mask).

All matmuls are bf16 (fp32 PSUM accumulate). Engine assignment:
  PE   : projections, scores (row-tiled hh pairs run concurrently),
         attn@V' (ones-column gives the softmax denominator), out-proj
  ACT  : exp (exp table stays loaded) + stage-C PSUM->SBUF copies
  DVE  : Q/K PSUM->SBUF copies w/ bias, P = exp(s)*M multiply (2x bf16),
         z-row staging + reciprocal
  Pool : softmax normalize multiplies, V copies, z broadcast, memsets
Inputs arrive as a few large merged DMAs split across both HWDGE rings
(sync + scalar) to cut ring serialization; outputs alternate rings.
"""

import sys

sys.path.insert(0, "/opt/trn_rl_repo")

import ml_dtypes
import numpy as np

import concourse.bass as bass
import concourse.tile as tile
from concourse import bacc, mybir
from concourse.alu_op_type import AluOpType
from concourse.bass_utils import run_bass_kernel_spmd

B, S, D, H = 4, 1024, 1024, 16
DH = D // H          # 64
HPC = 8              # heads per core
N_CORES = 8
KT = S // 128        # 8 k tiles
DCH = D // 128       # 8 contraction chunks
LAMBDA_MAX, ALPHA, SPARSE_THRESHOLD = 10.0, 5.0, 1.0

F32 = mybir.dt.float32
BF16 = mybir.dt.bfloat16
EXP = mybir.ActivationFunctionType.Exp


def build_program():
    nc = bacc.Bacc("TRN2", target_bir_lowering=False, debug=False,
                   num_devices=N_CORES)

    d_xt = nc.dram_tensor("xt", [D, S], BF16, kind="ExternalInput").ap()
    d_wqt = nc.dram_tensor("wqt", [D, 512], BF16, kind="ExternalInput").ap()
    d_wkt = nc.dram_tensor("wkt", [D, 512], BF16, kind="ExternalInput").ap()
    d_wvt = nc.dram_tensor("wvt", [D, 512], BF16, kind="ExternalInput").ap()
    d_wot = nc.dram_tensor("wot", [512, D], BF16, kind="ExternalInput").ap()
    d_mtd = nc.dram_tensor("mtd", [S, 1024], BF16, kind="ExternalInput").ap()
    d_bq = nc.dram_tensor("bq", [128, 4], F32, kind="ExternalInput").ap()
    d_bk = nc.dram_tensor("bk", [128, 4], F32, kind="ExternalInput").ap()
    d_out = nc.dram_tensor("out", [D, S], F32, kind="ExternalOutput").ap()

    with tile.TileContext(nc) as tc:
        with (
            tc.tile_pool(name="const", bufs=1) as constp,
            tc.tile_pool(name="main", bufs=1) as mainp,
            tc.tile_pool(name="inp", bufs=1) as inp,
            tc.tile_pool(name="ptp", bufs=16) as ptp,
            tc.tile_pool(name="smallp", bufs=4) as smallp,
            tc.tile_pool(name="psS", bufs=3, space="PSUM") as psS,
            tc.tile_pool(name="psV", bufs=2, space="PSUM") as psV,
        ):
            bq_sb = constp.tile([128, 4], F32, tag="bq")
            nc.scalar.dma_start(bq_sb[:], d_bq[:])
            bk_sb = constp.tile([128, 4], F32, tag="bk")
            nc.scalar.dma_start(bk_sb[:], d_bk[:])

            # Persistent arrays.
            qt_sb = [mainp.tile([128, S], BF16, tag=f"qt{m}", name=f"qt{m}")
                     for m in range(4)]
            kt_sb = [mainp.tile([128, S], BF16, tag=f"kt{m}", name=f"kt{m}")
                     for m in range(4)]
            vp_sb = [mainp.tile([128, HPC * (DH + 1)], BF16, tag=f"vp{sb}",
                                name=f"vp{sb}") for sb in range(8)]
            at_sb = [mainp.tile([128, S], BF16, tag=f"at{m}", name=f"at{m}")
                     for m in range(4)]
            wot_sb = mainp.tile([128, 4 * D], BF16, tag="wot", name="wot")

            # Stage-A inputs: one SBUF tile per DMA piece so dependency
            # tracking stays piece-granular (a merged tile would gate the
            # first matmul on the LAST piece's DMA).
            xt_t = [inp.tile([128, S], BF16, tag=f"xt{j}", name=f"xt{j}")
                    for j in range(8)]
            w_t = {nm: [inp.tile([128, 2 * 512], BF16, tag=f"w{nm}{j}",
                                 name=f"w{nm}{j}") for j in range(4)]
                   for nm in ("q", "k", "v")}
            mtd_t = [inp.tile([128, 2 * 1024], BF16, tag=f"mtd{j}",
                              name=f"mtd{j}") for j in range(4)]

            def xt_ap(c, lo, hi):
                return xt_t[c][:, lo:hi]

            def w_ap(nm, c, lo, hi):
                return w_t[nm][c // 2][:, (c % 2) * 512 + lo:
                                       (c % 2) * 512 + hi]

            def mtd_ap(k, lo, hi):
                return mtd_t[k // 2][:, (k % 2) * 1024 + lo:
                                     (k % 2) * 1024 + hi]

            def chunked(dst, src, n, csz, ring, pieces):
                """DMA dram [n*128, csz] -> sbuf [128, n*csz] in `pieces`."""
                d3 = dst.rearrange("p (c s) -> p c s", s=csz)
                s3 = src.rearrange("(c p) s -> p c s", p=128)
                step = n // pieces
                for i in range(pieces):
                    sl = slice(i * step, (i + 1) * step)
                    ring.dma_start(d3[:, sl], s3[:, sl])

            # The 16 SDMA engines round-robin BOTH rings' packets, so any
            # traffic on the second ring steals bandwidth from the critical
            # first pieces. Put every input on the sync ring in strict
            # first-use order (qk(0) streams (xt_c, wq_c) pairs, then wk,
            # then S(0,0)'s multiplies need mtd, then V needs wv); only
            # wot (needed late) and half the outputs use the scalar ring.
            def piece(dst, src, csz, ring, lo, hi):
                d3 = dst.rearrange("p (c s) -> p c s", s=csz)
                s3 = src.rearrange("(c p) s -> p c s", p=128)
                ring.dma_start(d3[:], s3[:, lo:hi])

            for j in range(4):
                piece(xt_t[2 * j], d_xt, S, nc.sync, 2 * j, 2 * j + 1)
                piece(xt_t[2 * j + 1], d_xt, S, nc.sync,
                      2 * j + 1, 2 * j + 2)
                piece(w_t["q"][j], d_wqt, 512, nc.sync, 2 * j, 2 * j + 2)
            for j in range(4):
                piece(w_t["k"][j], d_wkt, 512, nc.sync, 2 * j, 2 * j + 2)
            for j in range(4):
                piece(mtd_t[j], d_mtd, 1024, nc.sync, 2 * j, 2 * j + 2)
            for j in range(4):
                piece(w_t["v"][j], d_wvt, 512, nc.sync, 2 * j, 2 * j + 2)
            piece(wot_sb, d_wot, D, nc.scalar, 0, 4)

            # Ones columns of vp (softmax denominator rows): set once.
            for sb in range(8):
                vp3 = vp_sb[sb].rearrange("p (h d) -> p h d", d=DH + 1)
                nc.gpsimd.memset(vp3[:, :, DH:DH + 1], 1.0)

            # ---- emission helpers ----
            def emit_qk(m):
                """Q^T,K^T head-pair m: psum[d'128, s1024]; DVE copy+bias."""
                for nm, dst, bias in (("q", qt_sb, bq_sb), ("k", kt_sb, bk_sb)):
                    pp = psS.tile([128, 1024], F32, tag="ps", name=f"pp{nm}")
                    for st in range(2):
                        for c in range(DCH):
                            nc.tensor.matmul(
                                pp[:, st * 512:(st + 1) * 512],
                                w_ap(nm, c, m * 128, (m + 1) * 128),
                                xt_ap(c, st * 512, (st + 1) * 512),
                                start=(c == 0), stop=(c == DCH - 1))
                    nc.vector.tensor_scalar_add(dst[m][:], pp[:],
                                                bias[:, m:m + 1])

            def emit_v(sb):
                """V block sb: psum[s128, dh512] -> vp (strided bf16 copy)."""
                pv = psV.tile([128, 512], F32, tag="pv", name="pv")
                for c in range(DCH):
                    nc.tensor.matmul(
                        pv[:],
                        xt_ap(c, sb * 128, (sb + 1) * 128),
                        w_ap("v", c, 0, 512),
                        start=(c == 0), stop=(c == DCH - 1))
                vp3 = vp_sb[sb].rearrange("p (h d) -> p h d", d=DH + 1)
                nc.vector.tensor_copy(
                    vp3[:, :, 0:DH],
                    pv[:].rearrange("p (h d) -> p h d", d=DH))

            def emit_scores(m, q):
                """Scores k-tiles for group (m,q): P^T = exp(K^T.T@Q^T) * M."""
                pts = {}
                for k in range(KT):
                    ps = psS.tile([128, 1024], F32, tag="ps", name="ps")
                    for hh in range(2):
                        nc.tensor.matmul(
                            ps[:, hh * 512:(hh + 1) * 512],
                            kt_sb[m][hh * 64:(hh + 1) * 64,
                                     k * 128:(k + 1) * 128],
                            qt_sb[m][hh * 64:(hh + 1) * 64,
                                     q * 512:(q + 1) * 512],
                            start=True, stop=True,
                            tile_position=(hh * 64, 0))
                    pt = ptp.tile([128, 1024], BF16, tag="pt")
                    nc.scalar.activation(pt[:], ps[:], EXP)
                    # M slice broadcast across the two hh halves (stride-0).
                    msl = mtd_ap(k, q * 512, (q + 1) * 512)
                    nc.vector.tensor_tensor(
                        pt[:].rearrange("p (two s) -> p two s", two=2),
                        pt[:].rearrange("p (two s) -> p two s", two=2),
                        msl.rearrange("p (one s) -> p one s",
                                      one=1).broadcast_to((128, 2, 512)),
                        AluOpType.mult)
                    pts[k] = pt
                return pts

            def emit_attnv(m, q, pts):
                """attn@V' for group (m,q) + normalize into at_sb (bf16)."""
                pos = []
                for hh in range(2):
                    h = m * 2 + hh
                    po = psV.tile([DH + 1, 512], F32, tag="pv",
                                  name=f"po{hh}")
                    for k in range(KT):
                        nc.tensor.matmul(
                            po[:],
                            vp_sb[k][:, h * (DH + 1):(h + 1) * (DH + 1)],
                            pts[k][:, hh * 512:(hh + 1) * 512],
                            start=(k == 0), stop=(k == KT - 1))
                    pos.append(po)
                for hh in range(2):
                    # reciprocal_approx mis-addresses partition-offset PSUM
                    # inputs; stage the z row at partition 0 in SBUF first.
                    zrow = smallp.tile([1, 512], F32, tag="zrow",
                                       name=f"zr{hh}")
                    nc.vector.tensor_copy(zrow[:], pos[hh][DH:DH + 1, :])
                    rec = smallp.tile([1, 512], F32, tag="rec",
                                      name=f"rc{hh}")
                    nc.vector.reciprocal_approx_fast(rec[:], zrow[:])
                    bc = smallp.tile([64, 512], F32, tag="bc",
                                     name=f"bc{hh}")
                    nc.gpsimd.partition_broadcast(bc[:], rec[:])
                    nc.vector.tensor_tensor(
                        at_sb[m][hh * 64:(hh + 1) * 64,
                                 q * 512:(q + 1) * 512],
                        pos[hh][0:DH, :], bc[:], AluOpType.mult)

            def emit_outproj(dt, q):
                """out^T tile: psum[d_out 128, s 512] -> SBUF -> DMA out."""
                pc = psS.tile([128, 1024], F32, tag="ps", name=f"pc{dt}")
                pcs = pc[:, 0:512]
                for mc in range(4):
                    nc.tensor.matmul(
                        pcs,
                        wot_sb[:, mc * D + dt * 128:mc * D + (dt + 1) * 128],
                        at_sb[mc][:, q * 512:(q + 1) * 512],
                        start=(mc == 0), stop=(mc == 3))
                ot = smallp.tile([128, 512], F32, tag="ot", name=f"ot{dt}")
                nc.scalar.copy(ot[:], pcs)
                ring = nc.sync if (dt + q) % 2 == 0 else nc.scalar
                ring.dma_start(
                    d_out[dt * 128:(dt + 1) * 128, q * 512:(q + 1) * 512],
                    ot[:])

            # ---- static schedule ----
            # Prologue: heads m0, first scores early so ACT starts streaming,
            # then V / remaining projections fill PE while ACT works.
            emit_qk(0)
            sc = emit_scores(0, 0)
            for sb in range(8):
                emit_v(sb)
            emit_qk(1)
            prev = ((0, 0), sc)

            # Steady state: groups q-major; AV(g-1) + filler behind S(g).
            order = [(1, 0), (2, 0), (3, 0), (0, 1), (1, 1), (2, 1), (3, 1)]
            fillers = {(1, 0): lambda: emit_qk(2), (2, 0): lambda: emit_qk(3)}
            # C(q0) tiles interleave into the q1 groups: at iteration g the
            # emit_attnv(prev) call has just retired AV(3,0), so all at[*][:,
            # 0:512] slices are written before any C(*,0) read.
            outq0 = [(0, 1), (1, 1), (2, 1), (3, 1)]
            for g in order:
                sc = emit_scores(*g)
                emit_attnv(prev[0][0], prev[0][1], prev[1])
                if g in fillers:
                    fillers[g]()
                if g in outq0:
                    i = outq0.index(g)
                    emit_outproj(2 * i, 0)
                    emit_outproj(2 * i + 1, 0)
                prev = (g, sc)
            emit_attnv(prev[0][0], prev[0][1], prev[1])
            for dt in range(8):
                emit_outproj(dt, 1)

    nc.compile()
    return nc


_prog = None


def _get_prog():
    global _prog
    if _prog is None:
        _prog = build_program()
    return _prog


def _host_prep(x, prior_mask, prior_indices, prior_index_mask, u_prev,
               Wq, bq, Wk, bk, Wv, bv, Wo, bo):
    f32 = np.float32
    bf16 = ml_dtypes.bfloat16
    x = np.asarray(x, f32)
    pm = np.asarray(prior_mask, bool)
    idx = np.asarray(prior_indices)
    pim = np.asarray(prior_index_mask, bool)
    u = np.asarray(u_prev, f32).reshape(B)
    Wq, Wk, Wv, Wo = (np.asarray(w, f32) for w in (Wq, Wk, Wv, Wo))
    bq, bk, bv, bo = (np.asarray(v, f32) for v in (bq, bk, bv, bo))

    scale = f32(1.0 / np.sqrt(DH))
    lam = (LAMBDA_MAX * np.exp(-ALPHA * u.astype(np.float64))).astype(f32)
    use_sparse = lam >= SPARSE_THRESHOLD

    # Multiplicative prior M^T [k, q] (device broadcasts per q-half).
    mtd_sparse = None
    if use_sparse.any():
        cnt = np.zeros((S, S + 1), f32)
        np.add.at(cnt, (np.arange(S)[:, None],
                        np.where(pim, idx, S).astype(np.int64)), 1.0)
        mtd_sparse = np.ascontiguousarray(cnt[:, :S].T).astype(bf16)

    mtds = []
    for b in range(B):
        if use_sparse[b]:
            mtds.append(mtd_sparse)
        else:
            mt = np.where(pm, f32(1.0), np.exp(-lam[b], dtype=f32)).T
            mtds.append(np.ascontiguousarray(mt).astype(bf16))

    in_maps = []
    for c in range(N_CORES):
        b = c // 2
        hg = c % 2
        hsl = slice(hg * 512, (hg + 1) * 512)
        in_maps.append({
            "xt": np.ascontiguousarray(x[b].T).astype(bf16),
            "wqt": np.ascontiguousarray((Wq[hsl] * scale).T).astype(bf16),
            "wkt": np.ascontiguousarray(Wk[hsl].T).astype(bf16),
            "wvt": np.ascontiguousarray(Wv[hsl].T).astype(bf16),
            "wot": np.ascontiguousarray(Wo[:, hsl].T).astype(bf16),
            "mtd": mtds[b],
            "bq": np.ascontiguousarray((bq[hsl] * scale).reshape(4, 128).T),
            "bk": np.ascontiguousarray(bk[hsl].reshape(4, 128).T),
        })
    return in_maps


def kernel(**inputs):
    in_maps = _host_prep(**inputs)
    nc = _get_prog()
    res = run_bass_kernel_spmd(nc, in_maps, core_ids=list(range(N_CORES)))
    # Constant bias terms pass through the attention average unchanged:
    # out += bo + bv @ Wo^T  (z-normalized ones-column makes bv exact).
    bv = np.asarray(inputs["bv"], np.float32)
    bo = np.asarray(inputs["bo"], np.float32)
    Wo = np.asarray(inputs["Wo"], np.float32)
    const_row = bo + bv @ Wo.T
    out = np.empty((B, S, D), np.float32)
    for b in range(B):
        pt = res.results[2 * b]["out"] + res.results[2 * b + 1]["out"]
        out[b] = pt.T + const_row
    return out


# revision 26
# speedup vs baseline: 1.0477x; 1.0477x over previous
"""Trainium2 Bass kernel for ClippingAttentionEngine.

Sharding: core c -> (batch b = c//2, head-group hg = c%2, 8 heads each).
Each core computes Q/K/V projections for its 8 heads, attention, and the
partial transposed output projection over its head slice; host sums the two
per-batch partials, transposes, and adds the constant bias terms
(bo + bv @ Wo^T -- the V bias passes through softmax averaging unchanged).

The per-sample sparse/dense branch is folded into a single dense-shaped
program via a MULTIPLICATIVE prior M (host-built, bf16):
  dense batch:  M[q,k] = pm[q,k] ? 1 : exp(-lambda)
  sparse batch: M[q,k] = multiplicity of key k in prior_indices[q] (masked
                slots excluded), so P = exp(s) * M reproduces the gathered
                sparse softmax exactly (duplicates included, 0 = exact mask).

All matmuls are bf16 (fp32 PSUM accumulate). Engine assignment:
  PE   : projections, scores (row-tiled hh pairs run concurrently),
         attn@V' (ones-column gives the softmax denominator), out-proj
  ACT  : exp (exp table stays loaded) + stage-C PSUM->SBUF copies
  DVE  : Q/K PSUM->SBUF copies w/ bias, P = exp(s)*M multiply (2x bf16),
         z-row staging + reciprocal
  Pool : softmax normalize multiplies, V copies, z broadcast, memsets
Inputs arrive as a few large merged DMAs split across both HWDGE rings
(sync + scalar) to cut ring serialization; outputs alternate rings.
"""

import sys

sys.path.insert(0, "/opt/trn_rl_repo")

import ml_dtypes
import numpy as np

import concourse.bass as bass
import concourse.tile as tile
from concourse import bacc, mybir
from concourse.alu_op_type import AluOpType
from concourse.bass_utils import run_bass_kernel_spmd

B, S, D, H = 4, 1024, 1024, 16
DH = D // H          # 64
HPC = 8              # heads per core
N_CORES = 8
KT = S // 128        # 8 k tiles
DCH = D // 128       # 8 contraction chunks
LAMBDA_MAX, ALPHA, SPARSE_THRESHOLD = 10.0, 5.0, 1.0

F32 = mybir.dt.float32
BF16 = mybir.dt.bfloat16
EXP = mybir.ActivationFunctionType.Exp


def build_program():
    nc = bacc.Bacc("TRN2", target_bir_lowering=False, debug=False,
                   num_devices=N_CORES)

    d_xt = nc.dram_tensor("xt", [D, S], BF16, kind="ExternalInput").ap()
    d_wqt = nc.dram_tensor("wqt", [D, 512], BF16, kind="ExternalInput").ap()
    d_wkt = nc.dram_tensor("wkt", [D, 512], BF16, kind="ExternalInput").ap()
    d_wvt = nc.dram_tensor("wvt", [D, 512], BF16, kind="ExternalInput").ap()
    d_wot = nc.dram_tensor("wot", [512, D], BF16, kind="ExternalInput").ap()
    d_mtd = nc.dram_tensor("mtd", [S, 1024], BF16, kind="ExternalInput").ap()
    d_bq = nc.dram_tensor("bq", [128, 4], F32, kind="ExternalInput").ap()
    d_bk = nc.dram_tensor("bk", [128, 4], F32, kind="ExternalInput").ap()
    d_out = nc.dram_tensor("out", [D, S], F32, kind="ExternalOutput").ap()

    with tile.TileContext(nc) as tc:
        with (
            tc.tile_pool(name="const", bufs=1) as constp,
            tc.tile_pool(name="main", bufs=1) as mainp,
            tc.tile_pool(name="inp", bufs=1) as inp,
            tc.tile_pool(name="ptp", bufs=16) as ptp,
            tc.tile_pool(name="smallp", bufs=4) as smallp,
            tc.tile_pool(name="psS", bufs=3, space="PSUM") as psS,
            tc.tile_pool(name="psV", bufs=2, space="PSUM") as psV,
        ):
            bq_sb = constp.tile([128, 4], F32, tag="bq")
            nc.scalar.dma_start(bq_sb[:], d_bq[:])
            bk_sb = constp.tile([128, 4], F32, tag="bk")
            nc.scalar.dma_start(bk_sb[:], d_bk[:])

            # Persistent arrays.
            qt_sb = [mainp.tile([128, S], BF16, tag=f"qt{m}", name=f"qt{m}")
                     for m in range(4)]
            kt_sb = [mainp.tile([128, S], BF16, tag=f"kt{m}", name=f"kt{m}")
                     for m in range(4)]
            vp_sb = [mainp.tile([128, HPC * (DH + 1)], BF16, tag=f"vp{sb}",
                                name=f"vp{sb}") for sb in range(8)]
            at_sb = [mainp.tile([128, S], BF16, tag=f"at{m}", name=f"at{m}")
                     for m in range(4)]
            wot_sb = mainp.tile([128, 4 * D], BF16, tag="wot", name="wot")

            # Stage-A inputs: one SBUF tile per DMA piece so dependency
            # tracking stays piece-granular (a merged tile would gate the
            # first matmul on the LAST piece's DMA).
            xt_t = [inp.tile([128, S], BF16, tag=f"xt{j}", name=f"xt{j}")
                    for j in range(8)]
            # Q/K weights split by head-pair m (qk(m) only reads its own
            # 128-column slice -- loading per-m unblocks qk(0) after 0.5 MB
            # instead of 2 MB). V weights stay chunk-major (moving operand).
            wqm_t = {nm: [inp.tile([128, DCH * 128], BF16, tag=f"w{nm}{m}",
                                   name=f"w{nm}{m}") for m in range(4)]
                     for nm in ("q", "k")}
            wv_t = [inp.tile([128, 2 * 512], BF16, tag=f"wv{j}",
                             name=f"wv{j}") for j in range(4)]
            mtd_t = [inp.tile([128, 2 * 1024], BF16, tag=f"mtd{j}",
                              name=f"mtd{j}") for j in range(4)]

            def xt_ap(c, lo, hi):
                return xt_t[c][:, lo:hi]

            def wqk_ap(nm, m, c):
                return wqm_t[nm][m][:, c * 128:(c + 1) * 128]

            def wv_ap(c):
                return wv_t[c // 2][:, (c % 2) * 512:(c % 2 + 1) * 512]

            def mtd_ap(k, lo, hi):
                return mtd_t[k // 2][:, (k % 2) * 1024 + lo:
                                     (k % 2) * 1024 + hi]

            def chunked(dst, src, n, csz, ring, pieces):
                """DMA dram [n*128, csz] -> sbuf [128, n*csz] in `pieces`."""
                d3 = dst.rearrange("p (c s) -> p c s", s=csz)
                s3 = src.rearrange("(c p) s -> p c s", p=128)
                step = n // pieces
                for i in range(pieces):
                    sl = slice(i * step, (i + 1) * step)
                    ring.dma_start(d3[:, sl], s3[:, sl])

            # The 16 SDMA engines round-robin BOTH rings' packets, so any
            # traffic on the second ring steals bandwidth from the critical
            # first pieces. Put every input on the sync ring in strict
            # first-use order (qk(0) streams (xt_c, wq_c) pairs, then wk,
            # then S(0,0)'s multiplies need mtd, then V needs wv); only
            # wot (needed late) and half the outputs use the scalar ring.
            def piece(dst, src, csz, ring, lo, hi):
                d3 = dst.rearrange("p (c s) -> p c s", s=csz)
                s3 = src.rearrange("(c p) s -> p c s", p=128)
                ring.dma_start(d3[:], s3[:, lo:hi])

            def wqk_dma(nm, m):
                src = (d_wqt if nm == "q" else d_wkt)
                s3 = src[:, m * 128:(m + 1) * 128].rearrange(
                    "(c p) w -> p c w", p=128)
                d3 = wqm_t[nm][m][:].rearrange("p (c w) -> p c w", w=128)
                nc.sync.dma_start(d3, s3)

            wqk_dma("q", 0)
            wqk_dma("k", 0)
            for j in range(8):
                piece(xt_t[j], d_xt, S, nc.sync, j, j + 1)
            piece(mtd_t[0], d_mtd, 1024, nc.sync, 0, 2)
            wqk_dma("q", 1)
            wqk_dma("k", 1)
            piece(mtd_t[1], d_mtd, 1024, nc.sync, 2, 4)
            for j in range(4):
                piece(wv_t[j], d_wvt, 512, nc.sync, 2 * j, 2 * j + 2)
            piece(mtd_t[2], d_mtd, 1024, nc.sync, 4, 6)
            piece(mtd_t[3], d_mtd, 1024, nc.sync, 6, 8)
            wqk_dma("q", 2)
            wqk_dma("k", 2)
            wqk_dma("q", 3)
            wqk_dma("k", 3)
            piece(wot_sb, d_wot, D, nc.scalar, 0, 4)

            # Ones columns of vp (softmax denominator rows): set once.
            for sb in range(8):
                vp3 = vp_sb[sb].rearrange("p (h d) -> p h d", d=DH + 1)
                nc.gpsimd.memset(vp3[:, :, DH:DH + 1], 1.0)

            # ---- emission helpers ----
            def emit_qk(m):
                """Q^T,K^T head-pair m: psum[d'128, s1024]; DVE copy+bias."""
                for nm, dst, bias in (("q", qt_sb, bq_sb), ("k", kt_sb, bk_sb)):
                    pp = psS.tile([128, 1024], F32, tag="ps", name=f"pp{nm}")
                    for st in range(2):
                        for c in range(DCH):
                            nc.tensor.matmul(
                                pp[:, st * 512:(st + 1) * 512],
                                wqk_ap(nm, m, c),
                                xt_ap(c, st * 512, (st + 1) * 512),
                                start=(c == 0), stop=(c == DCH - 1))
                    nc.vector.tensor_scalar_add(dst[m][:], pp[:],
                                                bias[:, m:m + 1])

            def emit_v(sb):
                """V block sb: psum[s128, dh512] -> vp (strided bf16 copy)."""
                pv = psV.tile([128, 512], F32, tag="pv", name="pv")
                for c in range(DCH):
                    nc.tensor.matmul(
                        pv[:],
                        xt_ap(c, sb * 128, (sb + 1) * 128),
                        wv_ap(c),
                        start=(c == 0), stop=(c == DCH - 1))
                vp3 = vp_sb[sb].rearrange("p (h d) -> p h d", d=DH + 1)
                nc.vector.tensor_copy(
                    vp3[:, :, 0:DH],
                    pv[:].rearrange("p (h d) -> p h d", d=DH))

            def emit_scores(m, q):
                """Scores k-tiles for group (m,q): P^T = exp(K^T.T@Q^T) * M."""
                pts = {}
                for k in range(KT):
                    ps = psS.tile([128, 1024], F32, tag="ps", name="ps")
                    for hh in range(2):
                        nc.tensor.matmul(
                            ps[:, hh * 512:(hh + 1) * 512],
                            kt_sb[m][hh * 64:(hh + 1) * 64,
                                     k * 128:(k + 1) * 128],
                            qt_sb[m][hh * 64:(hh + 1) * 64,
                                     q * 512:(q + 1) * 512],
                            start=True, stop=True,
                            tile_position=(hh * 64, 0))
                    pt = ptp.tile([128, 1024], BF16, tag="pt")
                    nc.scalar.activation(pt[:], ps[:], EXP)
                    # M slice broadcast across the two hh halves (stride-0).
                    msl = mtd_ap(k, q * 512, (q + 1) * 512)
                    nc.vector.tensor_tensor(
                        pt[:].rearrange("p (two s) -> p two s", two=2),
                        pt[:].rearrange("p (two s) -> p two s", two=2),
                        msl.rearrange("p (one s) -> p one s",
                                      one=1).broadcast_to((128, 2, 512)),
                        AluOpType.mult)
                    pts[k] = pt
                return pts

            def emit_attnv(m, q, pts):
                """attn@V' for group (m,q) + normalize into at_sb (bf16)."""
                pos = []
                for hh in range(2):
                    h = m * 2 + hh
                    po = psV.tile([DH + 1, 512], F32, tag="pv",
                                  name=f"po{hh}")
                    for k in range(KT):
                        nc.tensor.matmul(
                            po[:],
                            vp_sb[k][:, h * (DH + 1):(h + 1) * (DH + 1)],
                            pts[k][:, hh * 512:(hh + 1) * 512],
                            start=(k == 0), stop=(k == KT - 1))
                    pos.append(po)
                for hh in range(2):
                    # reciprocal_approx mis-addresses partition-offset PSUM
                    # inputs; stage the z row at partition 0 in SBUF first.
                    zrow = smallp.tile([1, 512], F32, tag="zrow",
                                       name=f"zr{hh}")
                    nc.vector.tensor_copy(zrow[:], pos[hh][DH:DH + 1, :])
                    rec = smallp.tile([1, 512], F32, tag="rec",
                                      name=f"rc{hh}")
                    nc.vector.reciprocal_approx_fast(rec[:], zrow[:])
                    bc = smallp.tile([64, 512], F32, tag="bc",
                                     name=f"bc{hh}")
                    nc.gpsimd.partition_broadcast(bc[:], rec[:])
                    nc.vector.tensor_tensor(
                        at_sb[m][hh * 64:(hh + 1) * 64,
                                 q * 512:(q + 1) * 512],
                        pos[hh][0:DH, :], bc[:], AluOpType.mult)

            def emit_outproj(dt, q):
                """out^T tile: psum[d_out 128, s 512] -> SBUF -> DMA out."""
                pc = psS.tile([128, 1024], F32, tag="ps", name=f"pc{dt}")
                pcs = pc[:, 0:512]
                for mc in range(4):
                    nc.tensor.matmul(
                        pcs,
                        wot_sb[:, mc * D + dt * 128:mc * D + (dt + 1) * 128],
                        at_sb[mc][:, q * 512:(q + 1) * 512],
                        start=(mc == 0), stop=(mc == 3))
                ot = smallp.tile([128, 512], F32, tag="ot", name=f"ot{dt}")
                nc.scalar.copy(ot[:], pcs)
                ring = nc.sync if (dt + q) % 2 == 0 else nc.scalar
                ring.dma_start(
                    d_out[dt * 128:(dt + 1) * 128, q * 512:(q + 1) * 512],
                    ot[:])

            # ---- static schedule ----
            # Prologue: heads m0, first scores early so ACT starts streaming,
            # then V / remaining projections fill PE while ACT works.
            emit_qk(0)
            sc = emit_scores(0, 0)
            for sb in range(8):
                emit_v(sb)
            emit_qk(1)
            prev = ((0, 0), sc)

            # Steady state: groups q-major; AV(g-1) + filler behind S(g).
            order = [(1, 0), (2, 0), (3, 0), (0, 1), (1, 1), (2, 1), (3, 1)]
            fillers = {(1, 0): lambda: emit_qk(2), (2, 0): lambda: emit_qk(3)}
            # C(q0) tiles interleave into the q1 groups: at iteration g the
            # emit_attnv(prev) call has just retired AV(3,0), so all at[*][:,
            # 0:512] slices are written before any C(*,0) read.
            outq0 = [(0, 1), (1, 1), (2, 1), (3, 1)]
            for g in order:
                sc = emit_scores(*g)
                emit_attnv(prev[0][0], prev[0][1], prev[1])
                if g in fillers:
                    fillers[g]()
                if g in outq0:
                    i = outq0.index(g)
                    emit_outproj(2 * i, 0)
                    emit_outproj(2 * i + 1, 0)
                prev = (g, sc)
            emit_attnv(prev[0][0], prev[0][1], prev[1])
            for dt in range(8):
                emit_outproj(dt, 1)

    nc.compile()
    return nc


_prog = None


def _get_prog():
    global _prog
    if _prog is None:
        _prog = build_program()
    return _prog


def _host_prep(x, prior_mask, prior_indices, prior_index_mask, u_prev,
               Wq, bq, Wk, bk, Wv, bv, Wo, bo):
    f32 = np.float32
    bf16 = ml_dtypes.bfloat16
    x = np.asarray(x, f32)
    pm = np.asarray(prior_mask, bool)
    idx = np.asarray(prior_indices)
    pim = np.asarray(prior_index_mask, bool)
    u = np.asarray(u_prev, f32).reshape(B)
    Wq, Wk, Wv, Wo = (np.asarray(w, f32) for w in (Wq, Wk, Wv, Wo))
    bq, bk, bv, bo = (np.asarray(v, f32) for v in (bq, bk, bv, bo))

    scale = f32(1.0 / np.sqrt(DH))
    lam = (LAMBDA_MAX * np.exp(-ALPHA * u.astype(np.float64))).astype(f32)
    use_sparse = lam >= SPARSE_THRESHOLD

    # Multiplicative prior M^T [k, q] (device broadcasts per q-half).
    mtd_sparse = None
    if use_sparse.any():
        cnt = np.zeros((S, S + 1), f32)
        np.add.at(cnt, (np.arange(S)[:, None],
                        np.where(pim, idx, S).astype(np.int64)), 1.0)
        mtd_sparse = np.ascontiguousarray(cnt[:, :S].T).astype(bf16)

    mtds = []
    for b in range(B):
        if use_sparse[b]:
            mtds.append(mtd_sparse)
        else:
            mt = np.where(pm, f32(1.0), np.exp(-lam[b], dtype=f32)).T
            mtds.append(np.ascontiguousarray(mt).astype(bf16))

    in_maps = []
    for c in range(N_CORES):
        b = c // 2
        hg = c % 2
        hsl = slice(hg * 512, (hg + 1) * 512)
        in_maps.append({
            "xt": np.ascontiguousarray(x[b].T).astype(bf16),
            "wqt": np.ascontiguousarray((Wq[hsl] * scale).T).astype(bf16),
            "wkt": np.ascontiguousarray(Wk[hsl].T).astype(bf16),
            "wvt": np.ascontiguousarray(Wv[hsl].T).astype(bf16),
            "wot": np.ascontiguousarray(Wo[:, hsl].T).astype(bf16),
            "mtd": mtds[b],
            "bq": np.ascontiguousarray((bq[hsl] * scale).reshape(4, 128).T),
            "bk": np.ascontiguousarray(bk[hsl].reshape(4, 128).T),
        })
    return in_maps


def kernel(**inputs):
    in_maps = _host_prep(**inputs)
    nc = _get_prog()
    res = run_bass_kernel_spmd(nc, in_maps, core_ids=list(range(N_CORES)))
    # Constant bias terms pass through the attention average unchanged:
    # out += bo + bv @ Wo^T  (z-normalized ones-column makes bv exact).
    bv = np.asarray(inputs["bv"], np.float32)
    bo = np.asarray(inputs["bo"], np.float32)
    Wo = np.asarray(inputs["Wo"], np.float32)
    const_row = bo + bv @ Wo.T
    out = np.empty((B, S, D), np.float32)
    for b in range(B):
        pt = res.results[2 * b]["out"] + res.results[2 * b + 1]["out"]
        out[b] = pt.T + const_row
    return out
